# revision 5
# baseline (speedup 1.0000x reference)
"""VQ codebook kernel for Trainium2 (8 NeuronCores, SPMD data-parallel).

Matches reference:
    zp = z.transpose(0,2,3,4,1); z_flat = zp.reshape(-1, 256)
    d = ||z||^2 + ||c||^2 - 2 z.c ; idx = argmin(d)
    z_q = codebook[idx]; loss = 2*mean((z_q - zp)^2); out = straight-through.

Device (per core, 4096 tokens = one batch element):
    scores s~ = z.c computed with float32r matmuls (fast PE path, ~6e-7 abs err),
    evacuated PSUM->SBUF as fp16 by the scalar engine, max-folded 8192->2048 on
    the vector engine, then Max8 + MaxIndex give the top-8 folded slots =
    32 candidate codebook indices per token.
Host:
    exact fp32 rescore of the 32 candidates (incl. the ||c||^2 term) picks the
    true argmin; gather z_q, loss, and layout restoration are cheap host ops.
The candidate margin was validated offline: with 2x-conservative f32r noise and
fp16 quantization the true argmin always ranks <= 2 in the coarse ordering.
"""
import sys
sys.path.insert(0, "/opt/trn_rl_repo")
import numpy as np

B, C, D3 = 8, 256, 16 * 16 * 16       # z: [B, C, 16,16,16]
K = 8192                              # codebook entries
N_CORES = 8
TOK_PER_CORE = D3                     # 4096 (= one batch element per core)
NTILES = TOK_PER_CORE // 128          # 32 token tiles
KBLK = 512                            # psum bank width (fp32)
NKBLK = K // KBLK                     # 16
FOLD = 4                              # 8192 -> 2048 via two pairwise max folds
MFOLD = K // FOLD                     # 2048

_RUNNER = None


def _build_runner():
    import concourse.tile as tile
    from concourse import bacc, mybir
    import jax
    from jax.sharding import Mesh, PartitionSpec
    from jax.experimental.shard_map import shard_map
    from concourse.bass2jax import (
        _bass_exec_p, install_neuronx_cc_hook, partition_id_tensor,
    )

    nc = bacc.Bacc("TRN2", target_bir_lowering=False, debug=False)
    zt_in = nc.declare_dram_parameter("zt", [C, TOK_PER_CORE], mybir.dt.float32r, isOutput=False)
    cbt_in = nc.declare_dram_parameter("cbt", [C, K], mybir.dt.float32r, isOutput=False)
    i8_out = nc.declare_dram_parameter("i8", [128, NTILES * 8], mybir.dt.uint16, isOutput=True)

    with tile.TileContext(nc) as tc:
        with tc.tile_pool(name="const", bufs=1) as const, \
             tc.tile_pool(name="s16p", bufs=2) as s16p, \
             tc.tile_pool(name="foldp", bufs=2) as foldp, \
             tc.tile_pool(name="smallp", bufs=2) as smallp, \
             tc.tile_pool(name="psum", bufs=8, space="PSUM") as psum:
            zt0 = const.tile([128, TOK_PER_CORE], mybir.dt.float32r, tag="zt0")
            zt1 = const.tile([128, TOK_PER_CORE], mybir.dt.float32r, tag="zt1")
            cb0 = const.tile([128, K], mybir.dt.float32r, tag="cb0")
            cb1 = const.tile([128, K], mybir.dt.float32r, tag="cb1")
            i8_acc = const.tile([128, NTILES * 8], mybir.dt.uint16, tag="i8a")
            nc.gpsimd.dma_start(zt0[:], zt_in[0:128, :])
            nc.gpsimd.dma_start(zt1[:], zt_in[128:256, :])
            nc.gpsimd.dma_start(cb0[:], cbt_in[0:128, :])
            nc.gpsimd.dma_start(cb1[:], cbt_in[128:256, :])

            for t in range(NTILES):
                ts = slice(t * 128, (t + 1) * 128)
                s16 = s16p.tile([128, K], mybir.dt.float16, tag="s16")
                for j in range(NKBLK):
                    js = slice(j * KBLK, (j + 1) * KBLK)
                    ps = psum.tile([128, KBLK], mybir.dt.float32, tag="ps")
                    nc.tensor.matmul(ps[:], zt0[:, ts], cb0[:, js], start=True, stop=False)
                    nc.tensor.matmul(ps[:], zt1[:, ts], cb1[:, js], start=False, stop=True)
                    nc.scalar.copy(s16[:, js], ps[:])
                m1 = foldp.tile([128, K // 2], mybir.dt.float16, tag="m1")
                nc.vector.tensor_max(m1[:], s16[:, : K // 2], s16[:, K // 2:])
                m2 = foldp.tile([128, MFOLD], mybir.dt.float16, tag="m2")
                nc.vector.tensor_max(m2[:], m1[:, :MFOLD], m1[:, MFOLD:])
                v8 = smallp.tile([128, 8], mybir.dt.float16, tag="v8")
                nc.vector.max(out=v8[:], in_=m2[:])
                nc.vector.max_index(
                    out=i8_acc[:, t * 8:(t + 1) * 8], in_max=v8[:], in_values=m2[:])

            nc.gpsimd.dma_start(i8_out[:], i8_acc[:])
    nc.compile()

    install_neuronx_cc_hook()
    in_names, out_names, out_avals, zero_outs = [], [], [], []
    partition_name = nc.partition_id_tensor.name if nc.partition_id_tensor else None
    for alloc in nc.m.functions[0].allocations:
        if not isinstance(alloc, mybir.MemoryLocationSet):
            continue
        name = alloc.memorylocations[0].name
        if alloc.kind == "ExternalInput":
            if name != partition_name:
                in_names.append(name)
        elif alloc.kind == "ExternalOutput":
            out_names.append(name)
            shape = tuple(alloc.tensor_shape)
            dtype = mybir.dt.np(alloc.dtype)
            out_avals.append(jax.core.ShapedArray(shape, dtype))
            zero_outs.append(np.zeros(shape, dtype))
    n_params = len(in_names)
    n_outs = len(out_avals)
    all_in_names = list(in_names) + list(out_names)
    if partition_name is not None:
        all_in_names.append(partition_name)

    def _body(*args):
        operands = list(args)
        if partition_name is not None:
            operands.append(partition_id_tensor())
        outs = _bass_exec_p.bind(
            *operands,
            out_avals=tuple(out_avals),
            in_names=tuple(all_in_names),
            out_names=tuple(out_names),
            lowering_input_output_aliases=(),
            sim_require_finite=True,
            sim_require_nnan=True,
            nc=nc,
        )
        return tuple(outs)

    devices = jax.devices()[:N_CORES]
    mesh = Mesh(np.asarray(devices), ("core",))
    jitted = jax.jit(
        shard_map(_body, mesh=mesh,
                  in_specs=(PartitionSpec("core"),) * (n_params + n_outs),
                  out_specs=(PartitionSpec("core"),) * n_outs,
                  check_rep=False),
        keep_unused=True,
    )

    def run(in_maps):
        per_core = [[np.asarray(m[n]) for n in in_names] for m in in_maps]
        concat_in = [
            np.concatenate([per_core[c][i] for c in range(N_CORES)], axis=0)
            for i in range(n_params)
        ]
        concat_zeros = [
            np.zeros((N_CORES * z.shape[0], *z.shape[1:]), z.dtype)
            for z in zero_outs
        ]
        outs = jitted(*concat_in, *concat_zeros)
        jax.block_until_ready(outs)
        return [
            {n: np.asarray(outs[i]).reshape(N_CORES, *out_avals[i].shape)[c]
             for i, n in enumerate(out_names)}
            for c in range(N_CORES)
        ]
    return run


def kernel(z, codebook):
    """z: [8, 256, 16, 16, 16] f32; codebook: [8192, 256] f32.
    Returns (out [8,256,16,16,16] f32, loss f32 scalar, idx [32768] int32)."""
    global _RUNNER
    z = np.asarray(z, dtype=np.float32)
    codebook = np.asarray(codebook, dtype=np.float32)
    if _RUNNER is None:
        _RUNNER = _build_runner()

    cbt = np.ascontiguousarray(codebook.T)                      # [256, 8192]
    in_maps = [
        {"zt": np.ascontiguousarray(z[m].reshape(C, TOK_PER_CORE)), "cbt": cbt}
        for m in range(N_CORES)
    ]
    results = _RUNNER(in_maps)

    # folded top-8 slots -> 32 candidate global indices per token
    i8 = np.concatenate(
        [r["i8"].reshape(128, NTILES, 8).transpose(1, 0, 2) for r in results],
        axis=0,
    ).reshape(-1, 8).astype(np.int64)                           # [32768, 8] in [0, 2048)
    cand = (i8[:, :, None] + np.arange(FOLD)[None, None, :] * MFOLD).reshape(-1, 8 * FOLD)
    cand.sort(axis=1)                                           # first-occurrence tie-break

    # Exact rescore mirrors the reference's fp32 arithmetic:
    #   d = ||z||^2 + ||c||^2 - 2 z.c, all in fp32 (the +||z||^2 term rounds d
    # at ulp(256) ~ 1.5e-5, which defines the argmin the reference actually
    # produces; verified 0/32768 flips vs the jax reference).
    zp = z.transpose(0, 2, 3, 4, 1).reshape(-1, C)              # [32768, 256] f32
    zn = (zp ** 2).sum(1).astype(np.float32)                    # fp32 ||z||^2
    n32 = (codebook ** 2).sum(1).astype(np.float32)             # fp32 ||c||^2
    NTOT = zp.shape[0]
    win = np.empty(NTOT, dtype=np.int64)
    CH = 4096
    for s0 in range(0, NTOT, CH):
        s1 = min(s0 + CH, NTOT)
        cnd = cand[s0:s1]                                       # [T, 32] ascending
        cvec = codebook[cnd]                                    # [T, 32, 256] f32
        dots = np.einsum("tc,tkc->tk", zp[s0:s1], cvec,
                         optimize=True).astype(np.float32)
        d = (zn[s0:s1, None] + n32[cnd]) - 2.0 * dots           # fp32, ref order
        win[s0:s1] = cnd[np.arange(s1 - s0), np.argmin(d, axis=1)]

    idx = win.astype(np.int32)
    z_q = codebook[idx]                                         # [32768, 256] f32
    # straight-through estimator, mimicking reference fp32 arithmetic
    out_flat = zp + (z_q - zp)
    out = out_flat.reshape(B, 16, 16, 16, C).transpose(0, 4, 1, 2, 3)
    out = np.ascontiguousarray(out, dtype=np.float32)

    diff = z_q.astype(np.float64) - zp.astype(np.float64)
    loss = np.float32(2.0 * np.mean(diff * diff))
    return out, loss, idx
